# revision 1
# baseline (speedup 1.0000x reference)
"""Fused multi-head attention on 8 TRN2 NeuronCores.

Problem: x[2,2048,1024] -> q,k,v = x@W.T+b (16 heads x 64), softmax(q k^T/8) v,
then out @ Wp.T + bp.

Sharding: data-parallel over batch (2) x tensor-parallel over heads (4 ranks x
4 heads = 256 dims, Megatron-style).  Core c handles batch c//4, head-rank c%4.
The proj partial sums are reduced on the host (numpy), and the v-bias and
proj-bias are folded into one host-side vector bp_eff = bv @ Wp.T + bp.

Per-core layouts (host pre-transposes/pre-tiles, all DMA rows are >=2KB
contiguous DRAM runs):
  xT  [1024, 2048]  x[b].T
  wqT/wkT/wvT [128, 8*256]  W.T slice pre-tiled so partition p holds all 8
                            contraction tiles contiguously
  wpT [256, 1024]           Wp.T rows for this rank's 256 dims
  bq/bk [256, 1]
  outT [1024, 2048] partial (x[b] @ ..).T, missing bv/bp contributions

Kernel math per core (all matmul operands bfloat16, fp32 PSUM accumulate):
  qT = wqT.T @ xT + bq   [256, 2048]  (transposed layout, d on partitions)
  kT = wkT.T @ xT + bk   [256, 2048]
  v  = xT.T @ wvT        [2048, 256]  (natural layout, packed per head)
  attention runs as ONE flat software-pipelined stream over all 128
  (n-chunk, head-pair, key-block) blocks; per block:
     sT[m, n] = kT.T @ qT    two heads row-packed in the PE (K=64 tiles)
     p = exp(sT / 8)         ACT, one [128,1024] instr, both heads
     po[d, n]  += v.T @ p    col-packed pair, heads at partitions 0:64/64:128
     pd[d', n] += 1.T @ p    col-packed all-ones pair = softmax denominators
                             replicated over the partitions that need them
  PV/den trail their block's exp by one position globally, so the in-order
  PE always has the next score matmuls queued while ACT runs exp.  The
  q/k/v projections and the output projection are emitted as "filler"
  generators pumped one slice per block inside the stream, absorbing the
  PE idle gaps of the ACT-paced phase.
  attnT = po * reciprocal_approx_fast(pd)    one DVE mul per head pair
  outT += wpT.T @ attnT    [1024, n-chunk] per chunk, staged via SBUF
"""

import numpy as np

DIM = 1024
N_TOK = 2048
N_HEADS_LOC = 4       # heads per core
D_LOC = 256           # local q/k/v dims per core
SCALE = 64 ** -0.5
P = 128
CH = 512              # n-chunk (moving free dim)
NCH = N_TOK // CH     # 4
KT = DIM // P         # 8 contraction tiles for qkv/proj
MB = N_TOK // P       # 16 key blocks
N_CORES = 8

_NC_CACHE = {}


def build_nc(dt_mm_name="float32r"):
    import concourse.mybir as mybir
    import concourse.tile as tile
    from concourse import bacc
    from concourse.bass import ts

    f32 = mybir.dt.float32
    dt_mm = getattr(mybir.dt, dt_mm_name)
    Exp = mybir.ActivationFunctionType.Exp

    nc = bacc.Bacc("TRN2", target_bir_lowering=False, debug=False,
                   num_devices=N_CORES)
    xT = nc.dram_tensor("xT", [DIM, N_TOK], dt_mm, kind="ExternalInput").ap()
    wqT = nc.dram_tensor("wqT", [P, KT * D_LOC], dt_mm, kind="ExternalInput").ap()
    wkT = nc.dram_tensor("wkT", [P, KT * D_LOC], dt_mm, kind="ExternalInput").ap()
    wvT = nc.dram_tensor("wvT", [P, KT * D_LOC], dt_mm, kind="ExternalInput").ap()
    wpT = nc.dram_tensor("wpT", [D_LOC, DIM], dt_mm, kind="ExternalInput").ap()
    bq = nc.dram_tensor("bq", [D_LOC, 1], f32, kind="ExternalInput").ap()
    bk = nc.dram_tensor("bk", [D_LOC, 1], f32, kind="ExternalInput").ap()
    outT = nc.dram_tensor("outT", [DIM, N_TOK], f32, kind="ExternalOutput").ap()

    with tile.TileContext(nc) as tc:
        with (
            tc.tile_pool(name="const", bufs=1) as const,
            tc.tile_pool(name="work", bufs=2) as work,
            tc.tile_pool(name="psum", bufs=3, space="PSUM") as psum,
            tc.tile_pool(name="psum_o", bufs=2, space="PSUM") as psum_o,
        ):
            # ---- persistent SBUF state; issue order favors the first
            # ---- consumers (wk, then x, then the rest)
            w_tiles = {}
            for name in ("k", "q", "v"):
                w_tiles[name] = const.tile([P, KT, D_LOC], dt_mm, tag=f"w{name}",
                                           name=f"w{name}")
            for s in range(4):       # wk first, striped across queues
                nc.sync.dma_start(out=w_tiles["k"][ts(s, 32), :, :],
                                  in_=wkT[ts(s, 32), :].rearrange(
                                      "p (k n) -> p k n", k=KT))
            x_sb = []
            for i in range(KT):
                t = const.tile([P, N_TOK], dt_mm, tag=f"x{i}", name=f"x{i}")
                x_sb.append(t)
            for i in range(KT):      # gpsimd queue issues these in parallel;
                if i < 2:            # first tiles striped for an early start
                    for s in range(2):
                        nc.gpsimd.dma_start(
                            out=x_sb[i][ts(s, 64), :],
                            in_=xT[i * P + s * 64:i * P + (s + 1) * 64, :])
                else:
                    nc.gpsimd.dma_start(out=x_sb[i][:], in_=xT[ts(i, P), :])
            for name, src_ap in (("q", wqT), ("v", wvT)):
                for s in range(2):
                    nc.sync.dma_start(out=w_tiles[name][ts(s, 64), :, :],
                                      in_=src_ap[ts(s, 64), :].rearrange(
                                          "p (k n) -> p k n", k=KT))
            w_sb = {name: [w_tiles[name][:, i, :] for i in range(KT)]
                    for name in ("k", "q", "v")}
            bias_sb = {}
            for name, src_ap in (("q", bq), ("k", bk)):
                bias_sb[name] = []
                for mt in range(D_LOC // P):
                    t = const.tile([P, 1], f32, tag=f"b{name}{mt}",
                                   name=f"b{name}{mt}")
                    nc.sync.dma_start(out=t[:], in_=src_ap[ts(mt, P), :])
                    bias_sb[name].append(t)
            wp_sb = []
            for i in range(D_LOC // P):
                t = const.tile([P, DIM], dt_mm, tag=f"wp{i}", name=f"wp{i}")
                nc.sync.dma_start(out=t[:], in_=wpT[ts(i, P), :])
                wp_sb.append(t)

            ones_sb = const.tile([P, 64], dt_mm, tag="ones")
            nc.vector.memset(ones_sb[:], 1.0)

            qk_sb = {}
            for name in ("q", "k"):
                qk_sb[name] = [
                    const.tile([P, N_TOK], dt_mm, tag=f"{name}T{mt}",
                               name=f"{name}T{mt}")
                    for mt in range(D_LOC // P)
                ]
            vpk_sb = [
                const.tile([P, N_HEADS_LOC, 64], dt_mm, tag=f"vp{nt}",
                           name=f"vp{nt}")
                for nt in range(MB)
            ]
            at_sb = {}

            # ---- emission units; generators double as pipeline fillers ----
            def gen_proj(name, mt, h2, step):
                """q/k projection group; yields every `step` matmuls."""
                ps = psum.tile([P, 1024], f32, tag="ps", name=f"ps_{name}{mt}{h2}")
                n = 0
                for half in range(2):
                    for kt in range(KT):
                        nc.tensor.matmul(
                            ps[:, ts(half, CH)],
                            lhsT=w_sb[name][kt][:, ts(mt, P)],
                            rhs=x_sb[kt][:, ts(2 * h2 + half, CH)],
                            start=(kt == 0), stop=(kt == KT - 1),
                        )
                        n += 1
                        if n % step == 0:
                            yield
                    # per-half bias epilogue: downstream QK consumers wait
                    # on 512-col writes instead of the whole 1024-col group
                    nc.vector.tensor_scalar_add(
                        qk_sb[name][mt][:, ts(2 * h2 + half, CH)],
                        ps[:, ts(half, CH)], bias_sb[name][mt][:],
                    )
                yield

            def gen_vproj():
                """One v-projection group (one key block) per yield."""
                for nt in range(MB):
                    ps = psum.tile([P, 1024], f32, tag="ps", name=f"ps_v{nt}")
                    for kt in range(KT):
                        nc.tensor.matmul(
                            ps[:, 0:D_LOC],
                            lhsT=x_sb[kt][:, ts(nt, P)],
                            rhs=w_sb["v"][kt][:],
                            start=(kt == 0), stop=(kt == KT - 1),
                        )
                    for h in range(N_HEADS_LOC):
                        nc.vector.tensor_copy(vpk_sb[nt][:, h, :],
                                              ps[:, ts(h, 64)])
                    yield

            def gen_outproj(ch):
                """Output projection for chunk ch; one mo-pair per yield."""
                at_tiles = at_sb[ch]
                for mp in range(DIM // P // 2):
                    pp = psum.tile([P, 1024], f32, tag="ps", name=f"pp{ch}{mp}")
                    for half in range(2):
                        mo = 2 * mp + half
                        for dt_i in range(2):
                            nc.tensor.matmul(
                                pp[:, ts(half, CH)],
                                lhsT=wp_sb[dt_i][:, ts(mo, P)],
                                rhs=at_tiles[dt_i][:],
                                start=(dt_i == 0), stop=(dt_i == 1),
                            )
                    os_sb = work.tile([P, 1024], f32, tag="os", bufs=4,
                                      name=f"os{ch}{mp}")
                    nc.vector.tensor_copy(os_sb[:], pp[:])
                    for half in range(2):
                        mo = 2 * mp + half
                        nc.sync.dma_start(out=outT[ts(mo, P), ts(ch, CH)],
                                          in_=os_sb[:, ts(half, CH)])
                    yield

            def run(gen):
                for _ in gen:
                    pass

            # ---- flat software-pipelined stream over all key blocks ----
            # QK+exp lead PV/den by one block globally, so the in-order PE
            # always has score work queued while ACT runs exp, including
            # across (chunk, head-pair) boundaries.
            SEQ = [(0, 0), (1, 0), (0, 1), (1, 1),
                   (2, 0), (2, 1), (3, 0), (3, 1)]
            # per-position deferred PE work (must not be consumed in-loop,
            # except vproj/k01 whose consumers trail by the pipeline lag)
            from itertools import chain

            def pos0_filler():
                # one v-projection group per block (PV of block mb consumes
                # vpk[mb] one block later) plus a slice of the k h2=1 half
                # (its first QK consumer is block 8)
                vp = gen_vproj()
                kk = gen_proj("k", 0, 1, 3)
                for _ in range(MB):
                    next(vp, None)
                    next(kk, None)
                    yield

            fillers = {
                0: pos0_filler(),
                1: chain(gen_proj("k", 1, 0, 3), gen_proj("k", 1, 1, 3),
                         gen_proj("q", 1, 0, 3)),
                2: chain(gen_proj("q", 0, 1, 2), gen_proj("q", 1, 1, 2)),
                3: None,   # assigned below once at_sb[0] exists
                4: None,
                6: None,
            }

            run(gen_proj("k", 0, 0, 16))
            run(gen_proj("q", 0, 0, 16))

            blocks = [(i, c, h, mb) for i, (c, h) in enumerate(SEQ)
                      for mb in range(MB)]
            pts = {}
            pend = {}
            po_pd = {}
            for g in range(len(blocks) + 1):
                if g < len(blocks):
                    i, c, h, mb = blocks[g]
                    if mb == 0:
                        if i == 3:
                            fillers[3] = gen_outproj(0)
                        elif i == 4:
                            fillers[4] = gen_outproj(1)
                        elif i == 6:
                            fillers[6] = gen_outproj(2)
                        po_pd[(c, h)] = (
                            psum_o.tile([P, CH], f32, tag="po", name=f"po{c}{h}"),
                            psum_o.tile([P, CH], f32, tag="po", name=f"pd{c}{h}"),
                        )
                    f = fillers.get(i)
                    # outproj fillers wait one block for the preceding
                    # pair's normalize to be emitted
                    if f is not None and (i < 3 or mb >= 1):
                        next(f, None)
                    ps = psum.tile([P, 1024], f32, tag="ps", name=f"s{c}{h}{mb}")
                    nc.tensor.matmul(
                        ps[:, 0:CH],
                        lhsT=qk_sb["k"][h][0:64, ts(mb, P)],
                        rhs=qk_sb["q"][h][0:64, ts(c, CH)],
                    )
                    nc.tensor.matmul(
                        ps[:, CH:1024],
                        lhsT=qk_sb["k"][h][64:P, ts(mb, P)],
                        rhs=qk_sb["q"][h][64:P, ts(c, CH)],
                    )
                    pt = work.tile([P, 1024], dt_mm, tag="pt", bufs=8,
                                   name=f"pt{c}{h}{mb}")
                    nc.scalar.activation(pt[:], ps[:], Exp, scale=SCALE)
                    pts[(c, h, mb)] = pt
                    if mb == MB - 1 and f is not None:
                        run(f)   # drain deferred work before leaving position
                if g >= 1:
                    i2, c2, h2, mb2 = blocks[g - 1]
                    po, pd = po_pd[(c2, h2)]
                    pt = pts[(c2, h2, mb2)]
                    st = (mb2 == 0)
                    sp = (mb2 == MB - 1)
                    nc.tensor.matmul(
                        po[0:64, :], lhsT=vpk_sb[mb2][:, 2 * h2, :],
                        rhs=pt[:, 0:CH], start=st, stop=sp,
                    )
                    nc.tensor.matmul(
                        po[64:P, :], lhsT=vpk_sb[mb2][:, 2 * h2 + 1, :],
                        rhs=pt[:, CH:1024], start=st, stop=sp,
                    )
                    # denominators: adjacent exp tiles pair-summed on the
                    # gpsimd/DVE engines (alternating), denominator matmuls
                    # lag the adds by 2 blocks so the in-order PE never
                    # stalls on them; last two blocks use raw tiles
                    if mb2 % 2 == 1 and mb2 <= MB - 3:
                        pr = (mb2 - 1) // 2
                        pt0 = pts.pop((c2, h2, mb2 - 1))
                        pts2 = work.tile([P, 1024], dt_mm, tag="pts2", bufs=4,
                                         name=f"pts2_{c2}{h2}{mb2}")
                        eng = nc.gpsimd if pr % 2 == 0 else nc.vector
                        eng.tensor_add(pts2[:], pt0[:], pt[:])
                        pend.setdefault((c2, h2), []).append(pts2)
                        pts.pop((c2, h2, mb2))
                    if mb2 % 2 == 1 and mb2 >= 3:
                        s2 = pend[(c2, h2)].pop(0)
                        nc.tensor.matmul(
                            pd[0:64, :], lhsT=ones_sb[:],
                            rhs=s2[:, 0:CH], start=(mb2 == 3), stop=False,
                        )
                        nc.tensor.matmul(
                            pd[64:P, :], lhsT=ones_sb[:],
                            rhs=s2[:, CH:1024], start=(mb2 == 3), stop=False,
                        )
                    if sp:
                        for mbx in (MB - 2, MB - 1):
                            ptx = pts.pop((c2, h2, mbx))
                            nc.tensor.matmul(
                                pd[0:64, :], lhsT=ones_sb[:],
                                rhs=ptx[:, 0:CH], start=False,
                                stop=(mbx == MB - 1),
                            )
                            nc.tensor.matmul(
                                pd[64:P, :], lhsT=ones_sb[:],
                                rhs=ptx[:, CH:1024], start=False,
                                stop=(mbx == MB - 1),
                            )
                        # normalize: single reciprocal + multiply
                        del po_pd[(c2, h2)]
                        rec = work.tile([P, CH], f32, tag="bc", bufs=4,
                                        name=f"rec{c2}{h2}")
                        nc.vector.reciprocal_approx_fast(rec[:], pd[:])
                        at = work.tile([P, CH], dt_mm, tag="at", bufs=4,
                                       name=f"at{c2}{h2}")
                        nc.vector.tensor_mul(at[:], po[:], rec[:])
                        at_sb.setdefault(c2, []).append(at)
            run(gen_outproj(3))

    nc.compile()
    return nc


def _get_nc():
    if "nc" not in _NC_CACHE:
        _NC_CACHE["nc"] = build_nc(DT_MM_NAME)
    return _NC_CACHE["nc"]


def make_in_maps(x, Wq, bq, Wk, bk, Wv, bv, Wp, bp, dt_mm_name="float32r"):
    """Shard full inputs into 8 per-core input maps."""
    f = np.float32
    if dt_mm_name == "bfloat16":
        import ml_dtypes
        mmt = ml_dtypes.bfloat16
    else:
        mmt = np.float32
    x = np.asarray(x, f)
    xT = [np.ascontiguousarray(x[b].T).astype(mmt) for b in range(x.shape[0])]
    WqT = np.asarray(Wq, f).T
    WkT = np.asarray(Wk, f).T
    WvT = np.asarray(Wv, f).T
    WpT = np.asarray(Wp, f).T
    def pretile(w):
        # [1024, 256] -> [128, 8*256]: partition p holds all 8 k-tiles
        # contiguously so DMA descriptors are 4KB DRAM runs
        return np.ascontiguousarray(
            w.reshape(KT, P, D_LOC).transpose(1, 0, 2).reshape(P, KT * D_LOC)
        ).astype(mmt)

    in_maps = []
    for c in range(N_CORES):
        b, r = divmod(c, 4)
        sl = slice(D_LOC * r, D_LOC * (r + 1))
        in_maps.append({
            "xT": xT[b],
            "wqT": pretile(WqT[:, sl]),
            "wkT": pretile(WkT[:, sl]),
            "wvT": pretile(WvT[:, sl]),
            "wpT": np.ascontiguousarray(WpT[sl, :]).astype(mmt),
            "bq": np.asarray(bq, f)[sl].reshape(D_LOC, 1).copy(),
            "bk": np.asarray(bk, f)[sl].reshape(D_LOC, 1).copy(),
        })
    return in_maps


def assemble_output(results, Wv, bv, Wp, bp):
    """Sum TP partials, transpose back, add folded biases."""
    f = np.float32
    bp_eff = np.asarray(bv, f) @ np.asarray(Wp, f).T + np.asarray(bp, f)
    out = np.empty((2, N_TOK, DIM), f)
    for b in range(2):
        acc = results[4 * b][ "outT"].astype(f)
        for r in range(1, 4):
            acc = acc + results[4 * b + r]["outT"]
        out[b] = acc.T + bp_eff
    return out


DT_MM_NAME = "bfloat16"


def kernel(x, Wq, bq, Wk, bk, Wv, bv, Wp, bp):
    from concourse.bass_utils import run_bass_kernel_spmd
    nc = _get_nc()
    in_maps = make_in_maps(x, Wq, bq, Wk, bk, Wv, bv, Wp, bp, DT_MM_NAME)
    res = run_bass_kernel_spmd(nc, in_maps, list(range(N_CORES)))
    return assemble_output(res.results, Wv, bv, Wp, bp)



# revision 4
# speedup vs baseline: 1.1374x; 1.1374x over previous
"""Fused multi-head attention on 8 TRN2 NeuronCores — v2 (rebalanced stream).

Problem: x[2,2048,1024] -> q,k,v = x@W.T+b (16 heads x 64), softmax(q k^T/8) v,
then out @ Wp.T + bp.

Sharding: data-parallel over batch (2) x tensor-parallel over heads (4 ranks x
4 heads = 256 dims, Megatron-style).  Core c handles batch c//4, head-rank c%4.
The proj partial sums are reduced on the host (numpy), and the v-bias and
proj-bias are folded into one host-side vector bp_eff = bv @ Wp.T + bp.

v2 structure (from trace analysis of v1):
  - The attention stream is ACT(exp)-throughput-bound (~1.2us per
    [128,1024] exp).  PE work per block (score pair + PV pair + den pair,
    all auto-tile_position-concurrent) is ~0.5us, so all projection work
    runs as deadline-scheduled fillers inside the stream.
  - Prelude: chunk-major x DMA (one descriptor per 512-token chunk) +
    minimal warmup (k/q chunk0 + v block0) so the exp stream starts ~10us
    in instead of ~39us.
  - PV trails exp by PVLAG blocks so the po/pd PSUM pool never stalls the
    exp stream at (c,h)-stream boundaries.
  - Tail: final out-proj chunk interleaves matmuls/copies/DMAs over the
    sync and scalar HWDGE queues.

Per-core layouts (host pre-transposes/pre-tiles):
  xTc [4, 1024, 512]  x[b].T in 512-token chunks (chunk-major)
  wqT/wkT/wvT [128, 8*256]  W.T slice pre-tiled: partition p holds all 8
                            contraction tiles contiguously
  wpT [256, 1024]           Wp.T rows for this rank's 256 dims
  bqk [128, 4]              (bq mt0, bq mt1, bk mt0, bk mt1) columns
  outT [1024, 2048]         partial (x[b] @ ..).T, fp32
"""

import numpy as np

DIM = 1024
N_TOK = 2048
N_HEADS_LOC = 4       # heads per core
D_LOC = 256           # local q/k/v dims per core
SCALE = 64 ** -0.5
P = 128
CH = 512              # token chunk (moving free dim)
NCH = N_TOK // CH     # 4
KT = DIM // P         # 8 contraction tiles
MB = N_TOK // P       # 16 key blocks
N_CORES = 8
PVLAG = 4             # PV trails exp by this many blocks

SEQ = [(0, 0), (1, 0), (0, 1), (1, 1), (2, 0), (2, 1), (3, 0), (3, 1)]

_NC_CACHE = {}


def build_nc(dt_mm_name="bfloat16"):
    import concourse.mybir as mybir
    import concourse.tile as tile
    from concourse import bacc
    from concourse.bass import ts

    f32 = mybir.dt.float32
    dt_mm = getattr(mybir.dt, dt_mm_name)
    Exp = mybir.ActivationFunctionType.Exp

    nc = bacc.Bacc("TRN2", target_bir_lowering=False, debug=False,
                   num_devices=N_CORES)
    xTc = nc.dram_tensor("xTc", [NCH, DIM, CH], dt_mm, kind="ExternalInput").ap()
    wqT = nc.dram_tensor("wqT", [P, KT * D_LOC], dt_mm, kind="ExternalInput").ap()
    wkT = nc.dram_tensor("wkT", [P, KT * D_LOC], dt_mm, kind="ExternalInput").ap()
    wvT = nc.dram_tensor("wvT", [P, KT * D_LOC], dt_mm, kind="ExternalInput").ap()
    wpT = nc.dram_tensor("wpT", [D_LOC, DIM], dt_mm, kind="ExternalInput").ap()
    bqk = nc.dram_tensor("bqk", [P, 4], f32, kind="ExternalInput").ap()
    outT = nc.dram_tensor("outT", [DIM, N_TOK], f32, kind="ExternalOutput").ap()

    with tile.TileContext(nc) as tc:
        with (
            tc.tile_pool(name="const", bufs=1) as const,
            tc.tile_pool(name="work", bufs=2) as work,
            tc.tile_pool(name="psum", bufs=3, space="PSUM") as psum,
            tc.tile_pool(name="psum_o", bufs=2, space="PSUM") as psum_o,
        ):
            # ---- persistent SBUF state ----
            w_tiles = {name: const.tile([P, KT, D_LOC], dt_mm, tag=f"w{name}",
                                        name=f"w{name}")
                       for name in ("k", "q", "v")}
            x_all = const.tile([P, KT, N_TOK], dt_mm, tag="xall", name="xall")
            bqk_sb = const.tile([P, 4], f32, tag="bqk", name="bqk")
            wp_sb = [const.tile([P, DIM], dt_mm, tag=f"wp{i}", name=f"wp{i}")
                     for i in range(D_LOC // P)]

            # DMA plan: sync HWDGE queue, dependency order.  One descriptor
            # per logical fetch; chunk c of x = [128p, 8kt, 512t].
            def x_chunk_dma(c):
                nc.sync.dma_start(
                    out=x_all[:, :, ts(c, CH)],
                    in_=xTc[c, :, :].rearrange("(k p) t -> p k t", p=P))

            nc.sync.dma_start(out=w_tiles["k"][:],
                              in_=wkT.rearrange("p (k n) -> p k n", k=KT))
            x_chunk_dma(0)
            nc.sync.dma_start(out=bqk_sb[:], in_=bqk)
            nc.sync.dma_start(out=w_tiles["q"][:],
                              in_=wqT.rearrange("p (k n) -> p k n", k=KT))
            nc.sync.dma_start(out=w_tiles["v"][:],
                              in_=wvT.rearrange("p (k n) -> p k n", k=KT))
            x_chunk_dma(1)
            x_chunk_dma(2)
            x_chunk_dma(3)
            for i in range(D_LOC // P):
                nc.sync.dma_start(out=wp_sb[i][:], in_=wpT[ts(i, P), :])

            w_sb = {name: [w_tiles[name][:, i, :] for i in range(KT)]
                    for name in ("k", "q", "v")}
            bias_sb = {"q": [bqk_sb[:, 0:1], bqk_sb[:, 1:2]],
                       "k": [bqk_sb[:, 2:3], bqk_sb[:, 3:4]]}

            ones_sb = const.tile([P, 64], dt_mm, tag="ones")
            nc.vector.memset(ones_sb[:], 1.0)

            qk_sb = {name: [const.tile([P, N_TOK], dt_mm, tag=f"{name}T{mt}",
                                       name=f"{name}T{mt}")
                            for mt in range(D_LOC // P)]
                     for name in ("q", "k")}
            vpk_sb = [const.tile([P, N_HEADS_LOC, 64], dt_mm, tag=f"vp{nt}",
                                 name=f"vp{nt}")
                      for nt in range(MB)]
            at_sb = {}

            # ---- filler step generators (one PE matmul per yield) ----
            def kq_step(name, mt, c):
                ps = psum.tile([P, 1024], f32, tag="ps",
                               name=f"ps_{name}{mt}{c}")
                for kt in range(KT):
                    nc.tensor.matmul(
                        ps[:, 0:CH],
                        lhsT=w_sb[name][kt][:, ts(mt, P)],
                        rhs=x_all[:, kt, ts(c, CH)],
                        start=(kt == 0), stop=(kt == KT - 1),
                    )
                    yield
                nc.vector.tensor_scalar_add(
                    qk_sb[name][mt][:, ts(c, CH)], ps[:, 0:CH],
                    bias_sb[name][mt])

            def v_step(nt):
                ps = psum.tile([P, 1024], f32, tag="ps", name=f"ps_v{nt}")
                for kt in range(KT):
                    nc.tensor.matmul(
                        ps[:, 0:D_LOC],
                        lhsT=x_all[:, kt, ts(nt, P)],
                        rhs=w_sb["v"][kt][:],
                        start=(kt == 0), stop=(kt == KT - 1),
                    )
                    yield
                nc.vector.tensor_copy(vpk_sb[nt][:], ps[:, 0:D_LOC])

            def out_step(c, mp, tail=False):
                pp = psum.tile([P, 1024], f32, tag="ps", name=f"pp{c}{mp}")
                at_tiles = at_sb[c]
                for half in range(2):
                    mo = 2 * mp + half
                    for dt_i in range(2):
                        nc.tensor.matmul(
                            pp[:, ts(half, CH)],
                            lhsT=wp_sb[dt_i][:, ts(mo, P)],
                            rhs=at_tiles[dt_i][:],
                            start=(dt_i == 0), stop=(dt_i == 1),
                        )
                        yield
                os_sb = work.tile([P, 1024], f32, tag="os", bufs=4,
                                  name=f"os{c}{mp}")
                nc.vector.tensor_copy(os_sb[:], pp[:])
                for half in range(2):
                    mo = 2 * mp + half
                    q = nc.scalar if (tail and half == 1) else nc.sync
                    q.dma_start(out=outT[ts(mo, P), ts(c, CH)],
                                in_=os_sb[:, ts(half, CH)])

            # ---- deadline-scheduled filler queue ----
            # (deadline_pos, earliest_pos, generator)
            fillers = []

            def add_filler(deadline, earliest, gen):
                fillers.append([deadline, earliest, gen])

            # k chunks for stream 0 (h=0): chunk j needed by block 4j.
            for j in (1, 2, 3):
                add_filler(4 * j - 1, j - 1, kq_step("k", 0, j))
            # v blocks: vpk[nt] consumed at position nt + PVLAG.
            for nt in range(1, MB):
                add_filler(nt + PVLAG - 2, max(0, nt // 4 - 1), v_step(nt))
            # q chunks for h=0 streams: chunk c needed at SEQ position of
            # (c, 0) -> block 16*i.
            for i, (c, h) in enumerate(SEQ):
                if (c, h) == (0, 0):
                    continue
                if h == 0:
                    add_filler(16 * i - 1, c - 1, kq_step("q", 0, c))
            # k/q mt1 for h=1 streams.
            first_h1 = min(i for i, (c, h) in enumerate(SEQ) if h == 1)
            for j in range(NCH):
                add_filler(16 * first_h1 - 9 + 2 * j, j, kq_step("k", 1, j))
            for i, (c, h) in enumerate(SEQ):
                if h == 1:
                    add_filler(16 * i - 1, c, kq_step("q", 1, c))
            fillers.sort(key=lambda f: f[0])

            out_ready = {}   # c -> position when at_sb[c] complete
            for i, (c, h) in enumerate(SEQ):
                if h == 1:
                    out_ready[c] = 16 * i + 16 + PVLAG + 2

            def pump(gen, n=None):
                if n is None:
                    for _ in gen:
                        pass
                    return False
                for _ in range(n):
                    if next(gen, "END") == "END":
                        return False
                return True

            # ---- prelude: minimal warmup for the exp stream ----
            pump(kq_step("k", 0, 0))
            pump(kq_step("q", 0, 0))
            pump(v_step(0))

            # ---- the stream ----
            blocks = [(i, c, h, mb) for i, (c, h) in enumerate(SEQ)
                      for mb in range(MB)]
            NB = len(blocks)
            pts = {}          # position -> exp tile
            pend = {}         # (c,h) -> list of [created_pos, pair tile]
            po_pd = {}
            den_started = {}
            add_eng = [0]     # alternator for pair-add engine

            def emit_fillers(g):
                # anything at deadline: finish it now; else trickle.
                budget = 4 if g < 20 else (3 if g < 48 else 2)
                while fillers:
                    dl, ea, gen = fillers[0]
                    if dl <= g + 1:
                        pump(gen)
                        fillers.pop(0)
                        continue
                    if ea > g:
                        break
                    if budget <= 0:
                        break
                    if not pump(gen, budget):
                        fillers.pop(0)
                    budget = 0

            def emit_outproj(g):
                for c, rdy in list(out_ready.items()):
                    if c in at_sb and len(at_sb[c]) == 2 and g >= rdy:
                        del out_ready[c]
                        for mp_i in range(4):
                            add_filler(g + 4 * mp_i + 6, g, out_step(c, mp_i))
                        fillers.sort(key=lambda f: f[0])

            for g in range(NB + PVLAG):
                if g < NB:
                    emit_outproj(g)
                    emit_fillers(g)
                    i, c, h, mb = blocks[g]
                    ps = psum.tile([P, 1024], f32, tag="ps", name=f"s{c}{h}{mb}")
                    nc.tensor.matmul(
                        ps[:, 0:CH],
                        lhsT=qk_sb["k"][h][0:64, ts(mb, P)],
                        rhs=qk_sb["q"][h][0:64, ts(c, CH)],
                    )
                    nc.tensor.matmul(
                        ps[:, CH:1024],
                        lhsT=qk_sb["k"][h][64:P, ts(mb, P)],
                        rhs=qk_sb["q"][h][64:P, ts(c, CH)],
                    )
                    pt = work.tile([P, 1024], dt_mm, tag="pt", bufs=8,
                                   name=f"pt{c}{h}{mb}")
                    nc.scalar.activation(pt[:], ps[:], Exp, scale=SCALE)
                    pts[g] = pt

                gp = g - PVLAG
                if gp < 0:
                    continue
                i2, c2, h2, mb2 = blocks[gp]
                key = (c2, h2)
                if mb2 == 0:
                    po_pd[key] = (
                        psum_o.tile([P, CH], f32, tag="po", name=f"po{c2}{h2}"),
                        psum_o.tile([P, CH], f32, tag="po", name=f"pd{c2}{h2}"),
                    )
                    pend[key] = []
                    den_started[key] = False
                po, pd = po_pd[key]
                pt = pts[gp]
                st = (mb2 == 0)
                sp = (mb2 == MB - 1)
                nc.tensor.matmul(
                    po[0:64, :], lhsT=vpk_sb[mb2][:, 2 * h2, :],
                    rhs=pt[:, 0:CH], start=st, stop=sp,
                )
                nc.tensor.matmul(
                    po[64:P, :], lhsT=vpk_sb[mb2][:, 2 * h2 + 1, :],
                    rhs=pt[:, CH:1024], start=st, stop=sp,
                )
                # pair-sum of adjacent exp tiles (DVE/gpsimd alternating);
                # last pair (14,15) stays raw for the tail matmuls
                if mb2 % 2 == 1 and mb2 <= MB - 3:
                    pt0 = pts.pop(gp - 1)
                    pts2 = work.tile([P, 1024], dt_mm, tag="pts2", bufs=6,
                                     name=f"pts2_{c2}{h2}{mb2}")
                    # the last pair feeds the sp-drain soon after: keep it
                    # off gpsimd (2.5us add latency would stall the PE)
                    if mb2 >= MB - 3:
                        eng = nc.vector
                    else:
                        eng = nc.vector if add_eng[0] % 2 == 0 else nc.gpsimd
                        add_eng[0] += 1
                    eng.tensor_add(pts2[:], pt0[:], pt[:])
                    pend[key].append([g, pts2])
                    pts.pop(gp)
                # denominator matmul pair, lagged behind its pair-sum
                if pend[key] and pend[key][0][0] <= g - 2 and not sp:
                    _, s2 = pend[key].pop(0)
                    nc.tensor.matmul(
                        pd[0:64, :], lhsT=ones_sb[:], rhs=s2[:, 0:CH],
                        start=not den_started[key], stop=False,
                    )
                    nc.tensor.matmul(
                        pd[64:P, :], lhsT=ones_sb[:], rhs=s2[:, CH:1024],
                        start=not den_started[key], stop=False,
                    )
                    den_started[key] = True
                if sp:
                    # drain remaining pair-sums, then raw tails, then
                    # normalize
                    for _, s2 in pend.pop(key):
                        nc.tensor.matmul(
                            pd[0:64, :], lhsT=ones_sb[:], rhs=s2[:, 0:CH],
                            start=not den_started[key], stop=False,
                        )
                        nc.tensor.matmul(
                            pd[64:P, :], lhsT=ones_sb[:], rhs=s2[:, CH:1024],
                            start=not den_started[key], stop=False,
                        )
                        den_started[key] = True
                    for gx in (gp - 1, gp):
                        ptx = pts.pop(gx)
                        nc.tensor.matmul(
                            pd[0:64, :], lhsT=ones_sb[:], rhs=ptx[:, 0:CH],
                            start=False, stop=(gx == gp),
                        )
                        nc.tensor.matmul(
                            pd[64:P, :], lhsT=ones_sb[:], rhs=ptx[:, CH:1024],
                            start=False, stop=(gx == gp),
                        )
                    del po_pd[key]
                    rec = work.tile([P, CH], f32, tag="rec", bufs=4,
                                    name=f"rec{c2}{h2}")
                    nc.vector.reciprocal_approx_fast(rec[:], pd[:])
                    at = work.tile([P, CH], dt_mm, tag="at", bufs=4,
                                   name=f"at{c2}{h2}")
                    nc.vector.tensor_mul(at[:], po[:], rec[:])
                    at_sb.setdefault(c2, []).append(at)

            # ---- tail: final out-proj chunk ----
            emit_outproj(NB + PVLAG)
            for f in fillers:
                pump(f[2])
            fillers.clear()
            for mp in range(4):
                pump(out_step(3, mp, tail=True))

    nc.compile()
    return nc


def _get_nc():
    if "nc" not in _NC_CACHE:
        _NC_CACHE["nc"] = build_nc(DT_MM_NAME)
    return _NC_CACHE["nc"]


def make_in_maps(x, Wq, bq, Wk, bk, Wv, bv, Wp, bp, dt_mm_name="bfloat16"):
    """Shard full inputs into 8 per-core input maps."""
    f = np.float32
    if dt_mm_name == "bfloat16":
        import ml_dtypes
        mmt = ml_dtypes.bfloat16
    else:
        mmt = np.float32
    x = np.asarray(x, f)
    # chunk-major transposed x: [4 chunks, 1024 dims, 512 tokens]
    xTc = []
    for b in range(x.shape[0]):
        xt = np.ascontiguousarray(x[b].T)            # [1024, 2048]
        xTc.append(np.ascontiguousarray(
            xt.reshape(DIM, NCH, CH).transpose(1, 0, 2)).astype(mmt))
    WqT = np.asarray(Wq, f).T
    WkT = np.asarray(Wk, f).T
    WvT = np.asarray(Wv, f).T
    WpT = np.asarray(Wp, f).T

    def pretile(w):
        # [1024, 256] -> [128, 8*256]: partition p holds all 8 k-tiles
        return np.ascontiguousarray(
            w.reshape(KT, P, D_LOC).transpose(1, 0, 2).reshape(P, KT * D_LOC)
        ).astype(mmt)

    in_maps = []
    for core in range(N_CORES):
        b, r = divmod(core, 4)
        sl = slice(D_LOC * r, D_LOC * (r + 1))
        bq_l = np.asarray(bq, f)[sl]
        bk_l = np.asarray(bk, f)[sl]
        bqk_l = np.stack([bq_l[0:P], bq_l[P:2 * P],
                          bk_l[0:P], bk_l[P:2 * P]], axis=1)
        in_maps.append({
            "xTc": xTc[b],
            "wqT": pretile(WqT[:, sl]),
            "wkT": pretile(WkT[:, sl]),
            "wvT": pretile(WvT[:, sl]),
            "wpT": np.ascontiguousarray(WpT[sl, :]).astype(mmt),
            "bqk": np.ascontiguousarray(bqk_l).astype(f),
        })
    return in_maps


def assemble_output(results, Wv, bv, Wp, bp):
    """Sum TP partials, transpose back, add folded biases."""
    f = np.float32
    bp_eff = np.asarray(bv, f) @ np.asarray(Wp, f).T + np.asarray(bp, f)
    out = np.empty((2, N_TOK, DIM), f)
    for b in range(2):
        acc = results[4 * b]["outT"].astype(f)
        for r in range(1, 4):
            acc = acc + results[4 * b + r]["outT"]
        out[b] = acc.T + bp_eff
    return out


DT_MM_NAME = "bfloat16"


def kernel(x, Wq, bq, Wk, bk, Wv, bv, Wp, bp):
    from concourse.bass_utils import run_bass_kernel_spmd
    nc = _get_nc()
    in_maps = make_in_maps(x, Wq, bq, Wk, bk, Wv, bv, Wp, bp, DT_MM_NAME)
    res = run_bass_kernel_spmd(nc, in_maps, list(range(N_CORES)))
    return assemble_output(res.results, Wv, bv, Wp, bp)


# revision 9
# speedup vs baseline: 1.3286x; 1.1681x over previous
"""Fused multi-head attention on 8 TRN2 NeuronCores — v3.

Problem: x[2,2048,1024] -> q,k,v = x@W.T+b (16 heads x 64), softmax(q k^T/8) v,
then out @ Wp.T + bp.

Sharding: data-parallel over batch (2) x tensor-parallel over heads (4 ranks x
4 heads = 256 dims, Megatron-style).  Core c handles batch c//4, head-rank c%4.
The proj partial sums are reduced on the host (numpy); the v-bias and proj-bias
fold into one host-side vector bp_eff = bv @ Wp.T + bp.

v3 structure (trace-driven):
  - exp on ACT is the steady-state pace (~1.2us per [128,1024] block); all
    projection work runs as deadline-scheduled fillers under it.
  - PV trails exp by a lag schedule (16 early -> 4 late).  A long early lag
    postpones the v-projection + PV deadline pressure out of the congested
    stream head; the ramp-down keeps the tail short.  po/pd PSUM slots
    support any constant-or-decreasing lag (closeout of stream S is always
    emitted one position before stream S+1's first PV).
  - Denominators: exp tiles pair-summed, pairs quad-summed (blocks 0-11) on
    DVE/gpsimd, then ones-matmul pairs into pd (col-tiled concurrent).
  - x is fetched in kt-interleaved 512-token chunks (one descriptor, 8KB
    contiguous per partition); dummy matmuls warm the PE HAM clock during
    the DMA wait.
  - Output staged in bf16, one batched DMA per out-proj step; tail uses the
    scalar HWDGE queue alongside sync.
"""

import numpy as np

DIM = 1024
N_TOK = 2048
N_HEADS_LOC = 4       # heads per core
D_LOC = 256           # local q/k/v dims per core
SCALE = 64 ** -0.5
P = 128
CH = 512              # token chunk (moving free dim)
NCH = N_TOK // CH     # 4
KT = DIM // P         # 8 contraction tiles
MB = N_TOK // P       # 16 key blocks
N_CORES = 8

SEQ = [(0, 0), (1, 0), (0, 1), (1, 1), (2, 0), (2, 1), (3, 0), (3, 1)]

_NC_CACHE = {}


def build_nc(dt_mm_name="bfloat16"):
    import concourse.mybir as mybir
    import concourse.tile as tile
    from concourse import bacc
    from concourse.bass import ts

    f32 = mybir.dt.float32
    dt_mm = getattr(mybir.dt, dt_mm_name)
    Exp = mybir.ActivationFunctionType.Exp

    nc = bacc.Bacc("TRN2", target_bir_lowering=False, debug=False,
                   num_devices=N_CORES)
    xTc = nc.dram_tensor("xTc", [NCH, P, KT * CH], dt_mm,
                         kind="ExternalInput").ap()
    wqT = nc.dram_tensor("wqT", [P, KT * D_LOC], dt_mm, kind="ExternalInput").ap()
    wkT = nc.dram_tensor("wkT", [P, KT * D_LOC], dt_mm, kind="ExternalInput").ap()
    wvT = nc.dram_tensor("wvT", [P, KT * D_LOC], dt_mm, kind="ExternalInput").ap()
    wpT = nc.dram_tensor("wpT", [D_LOC, DIM], dt_mm, kind="ExternalInput").ap()
    bqk = nc.dram_tensor("bqk", [P, 4], f32, kind="ExternalInput").ap()
    outT = nc.dram_tensor("outT", [DIM, N_TOK], dt_mm, kind="ExternalOutput").ap()

    with tile.TileContext(nc) as tc:
        with (
            tc.tile_pool(name="const", bufs=1) as const,
            tc.tile_pool(name="work", bufs=2) as work,
            tc.tile_pool(name="psum", bufs=3, space="PSUM") as psum,
            tc.tile_pool(name="psum_o", bufs=2, space="PSUM") as psum_o,
        ):
            # ---- persistent SBUF state ----
            w_tiles = {name: const.tile([P, KT, D_LOC], dt_mm, tag=f"w{name}",
                                        name=f"w{name}")
                       for name in ("k", "q", "v")}
            x_all = const.tile([P, KT, N_TOK], dt_mm, tag="xall", name="xall")
            bqk_sb = const.tile([P, 4], f32, tag="bqk", name="bqk")
            wp_sb = [const.tile([P, DIM], dt_mm, tag=f"wp{i}", name=f"wp{i}")
                     for i in range(D_LOC // P)]

            def x_chunk_dma(c):
                nc.sync.dma_start(
                    out=x_all[:, :, ts(c, CH)],
                    in_=xTc[c, :, :].rearrange("p (k t) -> p k t", k=KT))

            x_chunk_dma(0)
            nc.sync.dma_start(out=w_tiles["k"][:],
                              in_=wkT.rearrange("p (k n) -> p k n", k=KT))
            nc.sync.dma_start(out=bqk_sb[:], in_=bqk)
            nc.sync.dma_start(out=w_tiles["q"][:],
                              in_=wqT.rearrange("p (k n) -> p k n", k=KT))
            nc.sync.dma_start(out=w_tiles["v"][:],
                              in_=wvT.rearrange("p (k n) -> p k n", k=KT))
            x_chunk_dma(1)
            x_chunk_dma(2)
            x_chunk_dma(3)
            for i in range(D_LOC // P):
                nc.sync.dma_start(out=wp_sb[i][:], in_=wpT[ts(i, P), :])

            w_sb = {name: [w_tiles[name][:, i, :] for i in range(KT)]
                    for name in ("k", "q", "v")}
            bias_sb = {"q": [bqk_sb[:, 0:1], bqk_sb[:, 1:2]],
                       "k": [bqk_sb[:, 2:3], bqk_sb[:, 3:4]]}

            ones_sb = const.tile([P, 64], dt_mm, tag="ones")
            nc.vector.memset(ones_sb[:], 1.0)

            # HAM warmup: keep the PE busy ~4us during the x DMA wait so the
            # projection matmuls run at 2.4GHz.  No data deps beyond ones_sb.
            wps = psum.tile([P, 1024], f32, tag="ps", name="warm")
            for _ in range(48):
                nc.tensor.matmul(wps[0:64, 0:64], lhsT=ones_sb[:, 0:64],
                                 rhs=ones_sb[:, 0:64])

            qk_sb = {name: [const.tile([P, N_TOK], dt_mm, tag=f"{name}T{mt}",
                                       name=f"{name}T{mt}")
                            for mt in range(D_LOC // P)]
                     for name in ("q", "k")}
            vpk_sb = [const.tile([P, N_HEADS_LOC, 64], dt_mm, tag=f"vp{nt}",
                                 name=f"vp{nt}")
                      for nt in range(MB)]
            at_sb = {}

            # ---- filler step generators (one PE matmul per yield) ----
            def kq_step(name, mt, c):
                ps = psum.tile([P, 1024], f32, tag="ps",
                               name=f"ps_{name}{mt}{c}")
                for kt in range(KT):
                    nc.tensor.matmul(
                        ps[:, 0:CH],
                        lhsT=w_sb[name][kt][:, ts(mt, P)],
                        rhs=x_all[:, kt, ts(c, CH)],
                        start=(kt == 0), stop=(kt == KT - 1),
                    )
                    yield
                nc.vector.tensor_scalar_add(
                    qk_sb[name][mt][:, ts(c, CH)], ps[:, 0:CH],
                    bias_sb[name][mt])

            def v_step(nt):
                ps = psum.tile([P, 1024], f32, tag="ps", name=f"ps_v{nt}")
                for kt in range(KT):
                    nc.tensor.matmul(
                        ps[:, 0:D_LOC],
                        lhsT=x_all[:, kt, ts(nt, P)],
                        rhs=w_sb["v"][kt][:],
                        start=(kt == 0), stop=(kt == KT - 1),
                    )
                    yield
                nc.vector.tensor_copy(vpk_sb[nt][:], ps[:, 0:D_LOC])

            def out_step(c, mp, tail=False):
                pp = psum.tile([P, 1024], f32, tag="ps", name=f"pp{c}{mp}")
                at_tiles = at_sb[c]
                for dt_i in range(2):
                    for half in range(2):
                        mo = 2 * mp + half
                        nc.tensor.matmul(
                            pp[:, ts(half, CH)],
                            lhsT=wp_sb[dt_i][:, ts(mo, P)],
                            rhs=at_tiles[dt_i][:],
                            start=(dt_i == 0), stop=(dt_i == 1),
                        )
                        yield
                os_sb = work.tile([P, 1024], dt_mm, tag="os", bufs=4,
                                  name=f"os{c}{mp}")
                nc.vector.tensor_copy(os_sb[:], pp[:])
                q = nc.scalar if (tail and mp % 2 == 1) else nc.sync
                q.dma_start(
                    out=outT[2 * mp * P:(2 * mp + 2) * P, ts(c, CH)].rearrange(
                        "(m p) t -> p m t", p=P),
                    in_=os_sb[:].rearrange("p (m t) -> p m t", m=2))

            # ---- deadline-scheduled filler queue ----
            fillers = []   # [deadline, earliest, generator]

            def add_filler(deadline, earliest, gen):
                fillers.append([deadline, earliest, gen])
                fillers.sort(key=lambda f: f[0])

            for j in (1, 2, 3):                      # k mt0 chunks
                add_filler(4 * j - 2, j - 1, kq_step("k", 0, j))
            add_filler(14, 0, kq_step("q", 0, 1))
            for nt in range(MB):                     # v blocks
                add_filler(nt + 12, max(0, (nt // 4) * 2), v_step(nt))
            for j in range(NCH):                     # k mt1 chunks
                add_filler(20 + 2 * j, j + 2, kq_step("k", 1, j))
            add_filler(30, 10, kq_step("q", 1, 0))
            add_filler(44, 20, kq_step("q", 1, 1))
            add_filler(60, 20, kq_step("q", 0, 2))
            add_filler(76, 40, kq_step("q", 1, 2))
            add_filler(92, 40, kq_step("q", 0, 3))
            add_filler(108, 60, kq_step("q", 1, 3))

            def pump(gen, n=None):
                if n is None:
                    for _ in gen:
                        pass
                    return False
                for _ in range(n):
                    if next(gen, "END") == "END":
                        return False
                return True

            def emit_fillers(g):
                budget = 5 if g < 32 else (3 if g < 48 else 2)
                while fillers:
                    dl, ea, gen = fillers[0]
                    if dl <= g + 1:
                        pump(gen)
                        fillers.pop(0)
                        continue
                    if ea > g or dl > g + 16 or budget <= 0:
                        break
                    if not pump(gen, budget):
                        fillers.pop(0)
                    budget = 0

            # ---- prelude: minimal warmup for the exp stream ----
            pump(kq_step("k", 0, 0))
            pump(kq_step("q", 0, 0))

            # ---- the stream ----
            blocks = [(i, c, h, mb) for i, (c, h) in enumerate(SEQ)
                      for mb in range(MB)]
            NB = len(blocks)
            pts = {}          # position -> exp tile
            pairs = {}        # (c,h) -> list of [pos, n_blocks, tile]
            pend = {}         # (c,h) -> list of [pos, tile] ready for pd
            po_pd = {}
            den_started = {}
            add_eng = [0]

            def lag_target(g):
                return 16 if g < 88 else max(4, 16 - (g - 88) // 2)

            def process_pv(gp, g):
                i2, c2, h2, mb2 = blocks[gp]
                key = (c2, h2)
                if mb2 == 0:
                    po_pd[key] = (
                        psum_o.tile([P, CH], f32, tag="po", name=f"po{c2}{h2}"),
                        psum_o.tile([P, CH], f32, tag="po", name=f"pd{c2}{h2}"),
                    )
                    pairs[key] = []
                    pend[key] = []
                    den_started[key] = False
                po, pd = po_pd[key]
                pt = pts[gp]
                st = (mb2 == 0)
                sp = (mb2 == MB - 1)
                nc.tensor.matmul(
                    po[0:64, :], lhsT=vpk_sb[mb2][:, 2 * h2, :],
                    rhs=pt[:, 0:CH], start=st, stop=sp,
                )
                nc.tensor.matmul(
                    po[64:P, :], lhsT=vpk_sb[mb2][:, 2 * h2 + 1, :],
                    rhs=pt[:, CH:1024], start=st, stop=sp,
                )
                # level-1 pair sums (blocks 14,15 stay raw for the drain)
                if mb2 % 2 == 1 and mb2 <= MB - 3:
                    pt0 = pts.pop(gp - 1)
                    ps2 = work.tile([P, 1024], dt_mm, tag="pts2", bufs=8,
                                    name=f"pts2_{c2}{h2}{mb2}")
                    if mb2 == MB - 3:
                        eng = nc.vector
                    else:
                        eng = nc.vector if add_eng[0] % 2 == 0 else nc.gpsimd
                        add_eng[0] += 1
                    eng.tensor_add(ps2[:], pt0[:], pt[:])
                    pts.pop(gp)
                    if mb2 <= 11:
                        pairs[key].append([g, 2, ps2])
                    else:
                        pend[key].append([g, ps2])
                # level-2 quad sums on DVE (inputs may be gpsimd-made; wait
                # 2 positions so their 2.5us latency never stalls DVE)
                if len(pairs[key]) >= 2 and pairs[key][1][0] <= g - 2:
                    g0, n0, t0 = pairs[key].pop(0)
                    g1, n1, t1 = pairs[key].pop(0)
                    qd = work.tile([P, 1024], dt_mm, tag="quad", bufs=4,
                                   name=f"qd{c2}{h2}{mb2}")
                    nc.vector.tensor_add(qd[:], t0[:], t1[:])
                    pend[key].append([g, qd])
                # denominator matmul pair, lagged behind its sum
                if pend[key] and pend[key][0][0] <= g - 2 and not sp:
                    _, s2 = pend[key].pop(0)
                    nc.tensor.matmul(
                        pd[0:64, :], lhsT=ones_sb[:], rhs=s2[:, 0:CH],
                        start=not den_started[key], stop=False,
                    )
                    nc.tensor.matmul(
                        pd[64:P, :], lhsT=ones_sb[:], rhs=s2[:, CH:1024],
                        start=not den_started[key], stop=False,
                    )
                    den_started[key] = True
                if sp:
                    for g0, n0, t0 in pairs.pop(key):
                        pend[key].append([g0, t0])
                    for _, s2 in pend.pop(key):
                        nc.tensor.matmul(
                            pd[0:64, :], lhsT=ones_sb[:], rhs=s2[:, 0:CH],
                            start=not den_started[key], stop=False,
                        )
                        nc.tensor.matmul(
                            pd[64:P, :], lhsT=ones_sb[:], rhs=s2[:, CH:1024],
                            start=not den_started[key], stop=False,
                        )
                        den_started[key] = True
                    for gx in (gp - 1, gp):
                        ptx = pts.pop(gx)
                        nc.tensor.matmul(
                            pd[0:64, :], lhsT=ones_sb[:], rhs=ptx[:, 0:CH],
                            start=False, stop=(gx == gp),
                        )
                        nc.tensor.matmul(
                            pd[64:P, :], lhsT=ones_sb[:], rhs=ptx[:, CH:1024],
                            start=False, stop=(gx == gp),
                        )
                    del po_pd[key]
                    rec = work.tile([P, CH], f32, tag="rec", bufs=4,
                                    name=f"rec{c2}{h2}")
                    nc.vector.reciprocal_approx_fast(rec[:], pd[:])
                    at = work.tile([P, CH], dt_mm, tag="at", bufs=4,
                                   name=f"at{c2}{h2}")
                    nc.vector.tensor_mul(at[:], po[:], rec[:])
                    at_sb.setdefault(c2, []).append(at)
                    if len(at_sb[c2]) == 2 and c2 < 3:
                        for mp_i in range(4):
                            add_filler(g + 4 * mp_i + 6, g,
                                       out_step(c2, mp_i))

            pv_done = 0
            for g in range(NB + 5):
                if g < NB:
                    emit_fillers(g)
                    i, c, h, mb = blocks[g]
                    ps = psum.tile([P, 1024], f32, tag="ps", name=f"s{c}{h}{mb}")
                    nc.tensor.matmul(
                        ps[:, 0:CH],
                        lhsT=qk_sb["k"][h][0:64, ts(mb, P)],
                        rhs=qk_sb["q"][h][0:64, ts(c, CH)],
                    )
                    nc.tensor.matmul(
                        ps[:, CH:1024],
                        lhsT=qk_sb["k"][h][64:P, ts(mb, P)],
                        rhs=qk_sb["q"][h][64:P, ts(c, CH)],
                    )
                    pt = work.tile([P, 1024], dt_mm, tag="pt", bufs=22,
                                   name=f"pt{c}{h}{mb}")
                    nc.scalar.activation(pt[:], ps[:], Exp, scale=SCALE)
                    pts[g] = pt
                while pv_done < NB and pv_done <= g - lag_target(g):
                    process_pv(pv_done, g)
                    pv_done += 1

            # ---- tail: final out-proj chunk ----
            for f in fillers:
                pump(f[2])
            fillers.clear()
            for mp in range(4):
                pump(out_step(3, mp, tail=True))

    nc.compile()
    return nc


def _get_nc():
    if "nc" not in _NC_CACHE:
        _NC_CACHE["nc"] = build_nc(DT_MM_NAME)
    return _NC_CACHE["nc"]


def make_in_maps(x, Wq, bq, Wk, bk, Wv, bv, Wp, bp, dt_mm_name="bfloat16"):
    """Shard full inputs into 8 per-core input maps."""
    f = np.float32
    if dt_mm_name == "bfloat16":
        import ml_dtypes
        mmt = ml_dtypes.bfloat16
    else:
        mmt = np.float32
    x = np.asarray(x, f)
    # kt-interleaved chunk-major x: [4 chunks, 128 partitions, 8*512] so one
    # DMA descriptor per chunk moves 8KB contiguous per partition.
    xTc = []
    for b in range(x.shape[0]):
        xt = np.ascontiguousarray(x[b].T)            # [1024, 2048]
        xTc.append(np.ascontiguousarray(
            xt.reshape(KT, P, NCH, CH).transpose(2, 1, 0, 3).reshape(
                NCH, P, KT * CH)).astype(mmt))
    WqT = np.asarray(Wq, f).T
    WkT = np.asarray(Wk, f).T
    WvT = np.asarray(Wv, f).T
    WpT = np.asarray(Wp, f).T

    def pretile(w):
        return np.ascontiguousarray(
            w.reshape(KT, P, D_LOC).transpose(1, 0, 2).reshape(P, KT * D_LOC)
        ).astype(mmt)

    in_maps = []
    for core in range(N_CORES):
        b, r = divmod(core, 4)
        sl = slice(D_LOC * r, D_LOC * (r + 1))
        bq_l = np.asarray(bq, f)[sl]
        bk_l = np.asarray(bk, f)[sl]
        bqk_l = np.stack([bq_l[0:P], bq_l[P:2 * P],
                          bk_l[0:P], bk_l[P:2 * P]], axis=1)
        in_maps.append({
            "xTc": xTc[b],
            "wqT": pretile(WqT[:, sl]),
            "wkT": pretile(WkT[:, sl]),
            "wvT": pretile(WvT[:, sl]),
            "wpT": np.ascontiguousarray(WpT[sl, :]).astype(mmt),
            "bqk": np.ascontiguousarray(bqk_l).astype(f),
        })
    return in_maps


def assemble_output(results, Wv, bv, Wp, bp):
    """Sum TP partials, transpose back, add folded biases."""
    f = np.float32
    bp_eff = np.asarray(bv, f) @ np.asarray(Wp, f).T + np.asarray(bp, f)
    out = np.empty((2, N_TOK, DIM), f)
    for b in range(2):
        acc = results[4 * b]["outT"].astype(f)
        for r in range(1, 4):
            acc = acc + results[4 * b + r]["outT"].astype(f)
        out[b] = acc.T + bp_eff
    return out


DT_MM_NAME = "bfloat16"


def kernel(x, Wq, bq, Wk, bk, Wv, bv, Wp, bp):
    from concourse.bass_utils import run_bass_kernel_spmd
    nc = _get_nc()
    in_maps = make_in_maps(x, Wq, bq, Wk, bk, Wv, bv, Wp, bp, DT_MM_NAME)
    res = run_bass_kernel_spmd(nc, in_maps, list(range(N_CORES)))
    return assemble_output(res.results, Wv, bv, Wp, bp)
